# revision 1
# baseline (speedup 1.0000x reference)
import sys
if '/opt/trn_rl_repo' not in sys.path:
    sys.path.insert(0, '/opt/trn_rl_repo')
"""Bass/Tile kernel builder for one transformer block, uniform SPMD program.

Each core receives row-permuted inputs (own TOWN rows first, then the other
rows of its batch element) plus data-driven causal masks, so all 8 cores run
the identical NEFF.  See npmodel.py for the numpy mirror.

Layouts (SBUF, partition dim first; KB figures are per-partition at full
size, budget 192 KB):
  hT   [128, CB, T2/2] f32r  h^T feature-major, one kv-half at a time (32 KB)
  QT   [128, NPAIR, TOWN] bf16 (16)  partitions 0-63 head 2p, 64-127 2p+1
  KT   [128, NPAIR, T2]   bf16 (32)  same partition split
  V    [128, NSB, H, 66]  bf16 (33)  token-major per s-block; v at cols
       0-63, ones at col 64 (every head).  AV matmuls accumulate [65, TT]
       at PSUM base 0 (d rows 0-63, denominator row 64); odd heads' rows
       hop to partitions 64-127 of attnT via a small SBUF->SBUF DMA.
  attnT[128, CB, TOWN] f32r (32)  normalized attention out, feature-major
  x2   [128, NTB, C]   f32  (32)  token-major residual stream
  rT   [128, FC, NTT, TT] bf16 (64)  relu(ffn1) feature-major
"""
from contextlib import ExitStack

import concourse.bass as bass
import concourse.mybir as mybir
import concourse.tile as tile
from concourse.masks import make_identity

F32 = mybir.dt.float32
F32R = mybir.dt.float32r
BF16 = mybir.dt.bfloat16
AF = mybir.ActivationFunctionType
ALU = mybir.AluOpType


class Cfg:
    def __init__(self, T2=2048, C=1024, H=16, F=None, eps=1e-5):
        self.T2, self.C, self.H = T2, C, H
        self.F = 4 * C if F is None else F
        self.HS = 64
        self.eps = eps
        self.TBLK = T2 // 4          # row block (ownL/ownH/otherL/otherH)
        self.TOWN = 2 * self.TBLK    # rows this core owns
        self.TT = self.TBLK          # t-tile width == block
        self.NTT = 2
        self.CB = C // 128
        self.NPAIR = H // 2
        self.NSB = T2 // 128         # s-blocks
        self.SBB = self.TBLK // 128  # s-blocks per row-block
        self.NTB = self.TOWN // 128  # own token-blocks
        self.FC = self.F // 128
        self.DH = min(H, 8)          # heads per V-proj chunk (N = DH*64)
        self.NCH = max(C // 512, 1)  # c_out chunks
        self.CHW = min(C, 512)       # c_out chunk width
        self.NST = self.T2 // self.TT   # kv tiles of width TT
        self.scale = C ** -0.5
        self.sched()                 # sets NMASK

    def sched(self):
        """Static attention schedule: list of (tt, sb, mask_idx|None)."""
        out = []
        mi = 0
        sbb = self.SBB
        ownL = list(range(0, sbb))
        ownH = list(range(sbb, 2 * sbb))
        othL = list(range(2 * sbb, 3 * sbb))
        othH = list(range(3 * sbb, 4 * sbb))
        for sb in ownL + othL:                     # ttA: diag + all-or-none
            out.append((0, sb, mi)); mi += 1
        for sb in ownL:
            out.append((1, sb, None))
        for sb in ownH:
            out.append((1, sb, mi)); mi += 1
        for sb in othL:
            out.append((1, sb, None))
        for sb in othH:
            out.append((1, sb, mi)); mi += 1
        self.NMASK = mi
        return out


def broadcast_ap(ap, parts=128):
    """[N] vector AP -> [parts, N] partition-broadcast AP (DMA source)."""
    return bass.AP(tensor=ap.tensor, offset=ap.offset,
                   ap=[[0, parts]] + list(ap.ap))


def build(nc, cfg: Cfg, debug=False, reps=1, stop_after=9):
    c = cfg
    sched = c.sched()
    NM = c.NMASK

    def din(name, shape, dt=F32):
        return nc.dram_tensor(name, shape, dt, kind="ExternalInput").ap()

    xp = din("xp", [c.T2, c.C])
    masks = din("masks", [NM, 128, c.TT], BF16)
    wq, bq = din("wq", [c.H, c.C, c.HS]), din("bq", [c.H, c.HS])
    wk, bk = din("wk", [c.H, c.C, c.HS]), din("bk", [c.H, c.HS])
    wv, bv = din("wv", [c.H, c.C, c.HS]), din("bv", [c.H, c.HS])
    wp, bp = din("wp", [c.C, c.C]), din("bp", [c.C])
    w1, b1 = din("w1", [c.C, c.F]), din("b1", [c.F])
    w2, b2 = din("w2", [c.F, c.C]), din("b2", [c.C])
    g1, be1 = din("g1", [c.C]), din("be1", [c.C])
    g2, be2 = din("g2", [c.C]), din("be2", [c.C])
    y = nc.dram_tensor("y", [c.TOWN, c.C], F32, kind="ExternalOutput").ap()

    dbg = {}
    if debug:
        for nm, shp, dt in [("d_hT", [128, c.CB, c.T2], F32),
                            ("d_QT", [128, c.NPAIR, c.TOWN], BF16),
                            ("d_KT", [128, c.NPAIR, c.T2], BF16),
                            ("d_V", [128, c.NSB, c.H, 66], BF16),
                            ("d_attnT", [128, c.CB, c.TOWN], F32),
                            ("d_x2", [128, c.NTB, c.C], F32),
                            ("d_rT", [128, c.FC, c.NTT, c.TT], BF16)]:
            dbg[nm] = nc.dram_tensor(nm, shp, dt, kind="ExternalOutput").ap()

    with tile.TileContext(nc) as tc:
     for _rep in range(reps):
      with ExitStack() as top:
        # ---- pool plan: two LIFO stacks realize the lifetime chain ------
        # LEFT:  consts | consts2(masks) | x2 | attnT(ph3-4) | rT(ph6-7)
        # RIGHT: qkv(QT/KT/V, ph1-3) | consts3(ph4-7) | h2T(ph5-6)
        consts = top.enter_context(tc.tile_pool(name="consts", bufs=1))
        es_qkv = ExitStack()
        qkvp = es_qkv.enter_context(
            tc.tile_pool(name="qkv", bufs=1, side="right"))

        g1b = consts.tile([128, c.C], F32, name="g1b")
        be1b = consts.tile([128, c.C], F32, name="be1b")
        bvb = consts.tile([128, c.C], F32, name="bvb")
        for t, src in [(g1b, g1), (be1b, be1), (bvb, bv.flatten())]:
            nc.sync.dma_start(out=t, in_=broadcast_ap(src))
        bqs = consts.tile([128, c.NPAIR], F32, name="bqs")
        nc.sync.dma_start(out=bqs,
                          in_=bq.rearrange("(pr two) d -> (two d) pr", two=2))
        bks = consts.tile([128, c.NPAIR], F32, name="bks")
        nc.sync.dma_start(out=bks,
                          in_=bk.rearrange("(pr two) d -> (two d) pr", two=2))
        ident = consts.tile([128, 128], F32, name="ident")
        make_identity(nc, ident)
        eps_t = consts.tile([128, 1], F32, name="eps")
        nc.vector.memset(eps_t, c.eps)
        ones64f = consts.tile([128, 64], F32, name="ones64f")
        nc.vector.memset(ones64f, 1.0)
        ones64 = consts.tile([128, 64], F32R, name="ones64")
        nc.vector.tensor_copy(out=ones64, in_=ones64f)

        QT = qkvp.tile([128, c.NPAIR, c.TOWN], BF16, name="QT")
        KT = qkvp.tile([128, c.NPAIR, c.T2], BF16, name="KT")
        V = qkvp.tile([128, c.NSB, c.H, 66], BF16, name="V")

        def ln_tile(pool, x_t, gb, bb, name):
            """LayerNorm one [128, C] token-major tile -> new sbuf tile."""
            n_sub = max(c.C // 512, 1)
            w_sub = min(c.C, 512)
            stats = pool.tile([128, n_sub, 6], F32, name=f"st_{name}")
            for i in range(n_sub):
                nc.vector.bn_stats(out=stats[:, i, :],
                                   in_=x_t[:, i * w_sub:(i + 1) * w_sub])
            mv = pool.tile([128, 2], F32, name=f"mv_{name}")
            nc.vector.bn_aggr(out=mv, in_=stats)
            rstd = pool.tile([128, 1], F32, name=f"rs_{name}")
            nc.scalar.activation(out=rstd, in_=mv[:, 1:2], func=AF.Sqrt,
                                 bias=eps_t, scale=1.0)
            nc.vector.reciprocal(out=rstd, in_=rstd)
            h_t = pool.tile([128, c.C], F32, name=f"h_{name}")
            nc.vector.tensor_scalar(out=h_t, in0=x_t, scalar1=mv[:, 0:1],
                                    scalar2=rstd, op0=ALU.subtract,
                                    op1=ALU.mult)
            nc.vector.tensor_tensor(out=h_t, in0=h_t, in1=gb, op=ALU.mult)
            nc.vector.tensor_tensor(out=h_t, in0=h_t, in1=bb, op=ALU.add)
            return h_t

        def transpose_to(pool_ps, h_t, dst, col0, name):
            """PE-transpose token-major [128, C] tile into feature-major
            dst[:, cb, col0:col0+128] (dst free layout [CB, cols])."""
            grp = 4 if c.CB % 4 == 0 else 1
            for g0 in range(0, c.CB, grp):
                pst = pool_ps.tile([128, grp * 128], F32, name=f"tp_{name}")
                for j in range(grp):
                    nc.tensor.transpose(
                        pst[:, j * 128:(j + 1) * 128],
                        h_t[:, (g0 + j) * 128:(g0 + j + 1) * 128], ident)
                out_view = dst[:, g0:g0 + grp, col0:col0 + 128]
                nc.vector.tensor_copy(
                    out=out_view,
                    in_=pst.rearrange("p (g t) -> p g t", g=grp))

        # ---- phase 1+2: LN1 + h^T + Q/K/V projections, per kv-half ------
        nc.vector.memset(V[:, :, :, 64:66], 0.0)
        nc.vector.memset(V[:, :, :, 64:65], 1.0)       # ones col (denom)
        bvv = bvb.rearrange("p (h d) -> p h d", d=64)
        wqv = wq.rearrange("(pr two) (cb p) d -> pr p cb two d", two=2, p=128)
        wkv = wk.rearrange("(pr two) (cb p) d -> pr p cb two d", two=2, p=128)
        wvv = wv.rearrange("h (cb p) d -> p cb h d", p=128)
        TH = c.T2 // 2               # kv-half width in positions
        for half in range(2):
            with ExitStack() as ph:
                p1 = ph.enter_context(tc.tile_pool(name="p1", bufs=2))
                ps1 = ph.enter_context(tc.tile_pool(name="ps1", bufs=2,
                                                    space="PSUM"))
                hTp = ph.enter_context(
                    tc.tile_pool(name="hT_pool", bufs=1, side="right"))
                wqk = ph.enter_context(
                    tc.tile_pool(name="wqk", bufs=2, side="right"))
                psqk = ph.enter_context(tc.tile_pool(name="psqk", bufs=2,
                                                     space="PSUM"))
                wvp = ph.enter_context(
                    tc.tile_pool(name="wvp", bufs=1, side="right"))
                NSL = TH // c.TT          # hT slabs this half
                TBS = c.TT // 128         # token-blocks per slab
                hT = [hTp.tile([128, c.CB, c.TT], F32R, name=f"hT{i}")
                      for i in range(NSL)]
                for tb in range(TH // 128):
                    pos0 = half * TH + tb * 128
                    x_t = p1.tile([128, c.C], F32, name="x_ln1")
                    nc.sync.dma_start(out=x_t, in_=xp[pos0:pos0 + 128, :])
                    h_t = ln_tile(p1, x_t, g1b, be1b, "ln1")
                    transpose_to(ps1, h_t, hT[tb // TBS], (tb % TBS) * 128,
                                 "h1")
                if debug:
                    for i in range(NSL):
                        lo = half * TH + i * c.TT
                        nc.sync.dma_start(
                            out=dbg["d_hT"][:, :, lo:lo + c.TT],
                            in_=hT[i].bitcast(F32))

                # --- Q (half 0 only) / K projections, per head pair ---
                for pr in range(c.NPAIR):
                    wq_t = None
                    if half == 0:
                        wq_t = wqk.tile([128, c.CB, 2, 64], F32R, name="wq_t")
                        for two in range(2):
                            nc.sync.dma_start(
                                out=wq_t[:, :, two, :],
                                in_=wqv[pr, :, :, two, :].bitcast(F32R))
                    wk_t = wqk.tile([128, c.CB, 2, 64], F32R, name="wk_t")
                    for two in range(2):
                        nc.sync.dma_start(
                            out=wk_t[:, :, two, :],
                            in_=wkv[pr, :, :, two, :].bitcast(F32R))
                    for st in range(NSL):
                        gsl = slice(half * TH + st * c.TT,
                                    half * TH + (st + 1) * c.TT)  # global
                        if half == 0:
                            pq = psqk.tile([128, c.TT], F32, name="pq")
                            for cb in range(c.CB):
                                nc.tensor.matmul(
                                    pq, wq_t[:, cb, :, :], hT[st][:, cb, :],
                                    start=(cb == 0), stop=(cb == c.CB - 1))
                            nc.vector.tensor_scalar(
                                out=QT[:, pr, gsl], in0=pq,
                                scalar1=bqs[:, pr:pr + 1], scalar2=None,
                                op0=ALU.add)
                        pk = psqk.tile([128, c.TT], F32, name="pk")
                        for cb in range(c.CB):
                            nc.tensor.matmul(
                                pk, wk_t[:, cb, :, :], hT[st][:, cb, :],
                                start=(cb == 0), stop=(cb == c.CB - 1))
                        nc.vector.tensor_scalar(
                            out=KT[:, pr, gsl], in0=pk,
                            scalar1=bks[:, pr:pr + 1], scalar2=None,
                            op0=ALU.add)

                # --- V projection (token-major, parity-split ones col) ---
                for hh in range(0, c.H, c.DH):
                    wv_t = wvp.tile([128, c.CB, c.DH, 64], F32R, name="wv_t")
                    for cb in range(c.CB):
                        nc.sync.dma_start(
                            out=wv_t[:, cb, :, :],
                            in_=wvv[:, cb, hh:hh + c.DH, :].bitcast(F32R))
                    for lsb in range(TH // 128):
                        sb = half * (TH // 128) + lsb
                        sl, co = lsb // TBS, (lsb % TBS) * 128
                        pv = psqk.tile([128, c.DH * 64], F32, name="pv")
                        for cb in range(c.CB):
                            nc.tensor.matmul(
                                pv, hT[sl][:, cb, co:co + 128],
                                wv_t[:, cb, :, :],
                                start=(cb == 0), stop=(cb == c.CB - 1))
                        nc.vector.tensor_tensor(
                            out=V[:, sb, hh:hh + c.DH, 0:64],
                            in0=pv.rearrange("p (h d) -> p h d", d=64),
                            in1=bvv[:, hh:hh + c.DH, :],
                            op=ALU.add)
        if debug:
            nc.sync.dma_start(out=dbg["d_QT"], in_=QT)
            nc.sync.dma_start(out=dbg["d_KT"], in_=KT)
            nc.sync.dma_start(out=dbg["d_V"], in_=V)

        if stop_after < 3:
            es_qkv.close()
            continue
        # ---- masks + reserved residual-stream space ---------------------
        consts2 = top.enter_context(tc.tile_pool(name="consts2", bufs=1))
        msk = consts2.tile([128, NM, c.TT], BF16, name="msk")
        nc.sync.dma_start(out=msk, in_=masks.rearrange("m p t -> p m t"))
        x2p = top.enter_context(tc.tile_pool(name="x2p", bufs=1))
        x2_sb = x2p.tile([128, c.NTB, c.C], F32, name="x2")
        es_att = ExitStack()
        attp = es_att.enter_context(tc.tile_pool(name="attp", bufs=1))
        attnT = attp.tile([128, c.CB, c.TOWN], F32R, name="attnT")

        # ---- phase 3: attention -----------------------------------------
        with ExitStack() as ph:
            att = ph.enter_context(tc.tile_pool(name="att", bufs=4))
            ps_s = ph.enter_context(tc.tile_pool(name="ps_s", bufs=2,
                                                 space="PSUM"))
            ps_prd = ph.enter_context(tc.tile_pool(name="ps_prd", bufs=1,
                                                   space="PSUM"))
            ps_av = ph.enter_context(tc.tile_pool(name="ps_av", bufs=2,
                                                  space="PSUM"))
            by_tt = {0: [], 1: []}
            for tt, sb, mi in sched:
                by_tt[tt].append((sb, mi))
            for tt in range(c.NTT):
                tsl = slice(tt * c.TT, (tt + 1) * c.TT)
                for pr in range(c.NPAIR):
                    for par in range(2):
                        hd = 2 * pr + par
                        rows = slice(par * 64, par * 64 + 64)
                        pav = ps_av.tile([128, c.TT], F32, name="pav")
                        avsl = slice(0, 65)
                        den_r = 64
                        steps = by_tt[tt]
                        # pair consecutive s-blocks: two matmuls into one
                        # 2-bank psum, one exp op, one (paired) mask mult
                        pairs = [steps[i:i + 2]
                                 for i in range(0, len(steps), 2)]
                        npairs = len(pairs)
                        for pi, pair in enumerate(pairs):
                            psc = ps_s.tile([128, len(pair), c.TT], F32,
                                            name="psc")
                            for j, (sb, mi) in enumerate(pair):
                                nc.tensor.matmul(
                                    psc[:, j, :],
                                    KT[rows, pr, sb * 128:(sb + 1) * 128],
                                    QT[rows, pr, tsl], start=True, stop=True)
                            pexp = att.tile([128, len(pair), c.TT], BF16,
                                            name="pexp")
                            nc.scalar.activation(out=pexp, in_=psc,
                                                 func=AF.Exp, scale=c.scale)
                            mis = [mi for (sb, mi) in pair]
                            if all(m is not None for m in mis):
                                nc.vector.tensor_tensor(
                                    out=pexp, in0=pexp,
                                    in1=msk[:, mis[0]:mis[0] + len(pair), :],
                                    op=ALU.mult)
                            else:
                                for j, m in enumerate(mis):
                                    if m is not None:
                                        nc.vector.tensor_tensor(
                                            out=pexp[:, j, :],
                                            in0=pexp[:, j, :],
                                            in1=msk[:, m, :], op=ALU.mult)
                            for j, (sb, mi) in enumerate(pair):
                                nc.tensor.matmul(
                                    pav[avsl, :], V[:, sb, hd, 0:65],
                                    pexp[:, j, :],
                                    start=(pi == 0 and j == 0),
                                    stop=(pi == npairs - 1
                                          and j == len(pair) - 1))
                        # reciprocal of the denominator row, then broadcast
                        # it to 64 partitions via a K=1 PE matmul with a
                        # ones column (gpsimd.partition_broadcast silently
                        # no-ops on this hardware path).
                        den = att.tile([128, c.TT], F32R, name="den")
                        nc.vector.tensor_copy(out=den[den_r:den_r + 1, :],
                                              in_=pav[den_r:den_r + 1, :])
                        with nc.allow_low_precision(
                                reason="fp32r rdenom for PE broadcast"):
                            nc.vector.reciprocal(
                                out=den[den_r:den_r + 1, :],
                                in_=den[den_r:den_r + 1, :])
                        prd = ps_prd.tile([128, c.TT], F32, name="prd")
                        nc.tensor.matmul(prd[0:64, :],
                                         ones64[den_r:den_r + 1, :],
                                         den[den_r:den_r + 1, :],
                                         start=True, stop=True)
                        rdb = att.tile([64, c.TT], F32, name="rdb")
                        nc.scalar.copy(out=rdb, in_=prd[0:64, :])
                        if par == 0:
                            nc.vector.tensor_tensor(
                                out=attnT[0:64, pr, tsl],
                                in0=pav[0:64, :], in1=rdb, op=ALU.mult)
                        else:
                            tmp = att.tile([64, c.TT], F32R, name="avtmp")
                            nc.vector.tensor_tensor(
                                out=tmp, in0=pav[0:64, :], in1=rdb,
                                op=ALU.mult)
                            nc.sync.dma_start(out=attnT[64:128, pr, tsl],
                                              in_=tmp)
            if debug:
                nc.sync.dma_start(out=dbg["d_attnT"], in_=attnT.bitcast(F32))

        es_qkv.close()       # free QT/KT/V (right stack)
        if stop_after < 4:
            es_att.close()
            continue

        # ---- late constants for phases 4-7 (right stack) ----------------
        consts3 = top.enter_context(
            tc.tile_pool(name="consts3", bufs=1, side="right"))
        g2b = consts3.tile([128, c.C], F32, name="g2b")
        be2b = consts3.tile([128, c.C], F32, name="be2b")
        bpb = consts3.tile([128, c.C], F32, name="bpb")
        b2b = consts3.tile([128, c.C], F32, name="b2b")
        for t, src in [(g2b, g2), (be2b, be2), (bpb, bp), (b2b, b2)]:
            nc.sync.dma_start(out=t, in_=broadcast_ap(src))
        b1s = consts3.tile([128, c.FC], F32, name="b1s")
        nc.sync.dma_start(out=b1s, in_=b1.rearrange("(fc p) -> p fc", p=128))

        # ---- phase 4: out-projection + residual -------------------------
        with ExitStack() as ph:
            p4 = ph.enter_context(tc.tile_pool(name="p4", bufs=3))
            ps4 = ph.enter_context(tc.tile_pool(name="ps4", bufs=3,
                                                space="PSUM"))
            wpp = ph.enter_context(
                tc.tile_pool(name="wpp", bufs=1, side="right"))
            wp_t = wpp.tile([128, c.CB, c.C], F32R, name="wp_t")
            nc.sync.dma_start(out=wp_t,
                              in_=wp.rearrange("(cb p) o -> p cb o",
                                               p=128).bitcast(F32R))
            for tb in range(c.NTB):
                x_t = p4.tile([128, c.C], F32, name="x_res")
                nc.sync.dma_start(out=x_t, in_=xp[tb * 128:(tb + 1) * 128, :])
                xb = p4.tile([128, c.C], F32, name="xb")
                nc.vector.tensor_tensor(out=xb, in0=x_t, in1=bpb, op=ALU.add)
                for ch in range(c.NCH):
                    csl = slice(ch * c.CHW, (ch + 1) * c.CHW)
                    pd = ps4.tile([128, c.CHW], F32, name="pd")
                    for cb in range(c.CB):
                        nc.tensor.matmul(
                            pd, attnT[:, cb, tb * 128:(tb + 1) * 128],
                            wp_t[:, cb, csl],
                            start=(cb == 0), stop=(cb == c.CB - 1))
                    nc.vector.tensor_tensor(out=x2_sb[:, tb, csl], in0=pd,
                                            in1=xb[:, csl], op=ALU.add)
            if debug:
                nc.sync.dma_start(out=dbg["d_x2"], in_=x2_sb)

        es_att.close()       # free attnT (left stack)
        if stop_after < 5:
            continue

        # ---- phase 5: LN2 + h2^T ----------------------------------------
        es_h2 = ExitStack()
        h2p = es_h2.enter_context(
            tc.tile_pool(name="h2p", bufs=1, side="right"))
        h2T = h2p.tile([128, c.CB, c.TOWN], F32R, name="h2T")
        with ExitStack() as ph:
            p5 = ph.enter_context(tc.tile_pool(name="p5", bufs=2))
            ps5 = ph.enter_context(tc.tile_pool(name="ps5", bufs=2,
                                                space="PSUM"))
            for tb in range(c.NTB):
                h2_t = ln_tile(p5, x2_sb[:, tb, :], g2b, be2b, "ln2")
                transpose_to(ps5, h2_t, h2T, tb * 128, "h2")

        if stop_after < 6:
            es_h2.close()
            continue
        # ---- phase 6: FFN1 (relu into rT) -------------------------------
        rtp = top.enter_context(tc.tile_pool(name="rtp", bufs=1))
        rT = rtp.tile([128, c.FC, c.NTT, c.TT], BF16, name="rT")
        with ExitStack() as ph:
            p6 = ph.enter_context(tc.tile_pool(name="p6", bufs=3))
            ps6 = ph.enter_context(tc.tile_pool(name="ps6", bufs=3,
                                                space="PSUM"))
            w1v = w1.rearrange("(cb p) f -> p cb f", p=128)
            for fc in range(c.FC):
                w1_t = p6.tile([128, c.CB, 128], F32R, name="w1_t")
                nc.sync.dma_start(
                    out=w1_t,
                    in_=w1v[:, :, fc * 128:(fc + 1) * 128].bitcast(F32R))
                for tt in range(c.NTT):
                    pf = ps6.tile([128, c.TT], F32, name="pf")
                    for cb in range(c.CB):
                        nc.tensor.matmul(
                            pf, w1_t[:, cb, :],
                            h2T[:, cb, tt * c.TT:(tt + 1) * c.TT],
                            start=(cb == 0), stop=(cb == c.CB - 1))
                    nc.scalar.activation(out=rT[:, fc, tt, :], in_=pf,
                                         func=AF.Relu,
                                         bias=b1s[:, fc:fc + 1])
            if debug:
                nc.sync.dma_start(out=dbg["d_rT"], in_=rT)

        es_h2.close()        # free h2T (right stack)
        if stop_after < 7:
            continue

        # ---- phase 7: FFN2 + residual + store ---------------------------
        with ExitStack() as ph:
            p7 = ph.enter_context(tc.tile_pool(name="p7", bufs=6))
            ps7 = ph.enter_context(tc.tile_pool(name="ps7", bufs=1,
                                                space="PSUM"))
            TTB = c.TT // 128        # token-blocks per t-tile
            for ch in range(c.NCH):
                csl = slice(ch * c.CHW, (ch + 1) * c.CHW)
                pos = [ps7.tile([128, c.CHW], F32, name=f"po{tb}")
                       for tb in range(c.NTB)]
                for fb in range(c.FC):
                    w2f = p7.tile([128, c.CHW], F32, name="w2f")
                    nc.sync.dma_start(out=w2f,
                                      in_=w2[fb * 128:(fb + 1) * 128, csl])
                    w2b = p7.tile([128, c.CHW], BF16, name="w2b")
                    nc.vector.tensor_copy(out=w2b, in_=w2f)
                    for tb in range(c.NTB):
                        tt, lo = tb // TTB, tb % TTB
                        nc.tensor.matmul(
                            pos[tb], rT[:, fb, tt, lo * 128:(lo + 1) * 128],
                            w2b, start=(fb == 0), stop=(fb == c.FC - 1))
                for tb in range(c.NTB):
                    ot = p7.tile([128, c.CHW], F32, name="ot")
                    nc.vector.tensor_tensor(out=ot, in0=pos[tb],
                                            in1=x2_sb[:, tb, csl],
                                            op=ALU.add)
                    nc.vector.tensor_tensor(out=ot, in0=ot, in1=b2b[:, csl],
                                            op=ALU.add)
                    nc.sync.dma_start(out=y[tb * 128:(tb + 1) * 128, csl],
                                      in_=ot)
    return nc


# ======================================================================
# Host side: shard full inputs across 8 cores, run the SPMD NEFF, gather.
# ======================================================================
import numpy as np
import ml_dtypes

_STATE = {}

W_NAMES = ['wq', 'bq', 'wk', 'bk', 'wv', 'bv', 'wp', 'bp', 'w1', 'b1',
           'w2', 'b2', 'g1', 'be1', 'g2', 'be2']


def core_perm(pid, T):
    """Row permutation for one core: [ownL, ownH, otherL, otherH] blocks.
    Cores 2b+0 own row-blocks (0, 3) of batch b; cores 2b+1 own (1, 2) —
    balanced causal attention load."""
    Tb = T // 4
    own, other = {0: ((0, 3), (1, 2)), 1: ((1, 2), (0, 3))}[pid]
    blocks = [own[0], own[1], other[0], other[1]]
    return np.concatenate([np.arange(b * Tb, (b + 1) * Tb) for b in blocks])


def build_masks_np(perm, T2, sched, nm):
    TT = T2 // 4
    masks = np.zeros((nm, 128, TT), np.float32)
    for tt, sb, mi in sched:
        if mi is None:
            continue
        tpos = np.arange(tt * TT, (tt + 1) * TT)
        spos = np.arange(sb * 128, (sb + 1) * 128)
        masks[mi] = (perm[spos][:, None] <= perm[tpos][None, :]).astype(
            np.float32)
    return masks


def get_compiled():
    if 'nc' in _STATE:
        return _STATE['nc'], _STATE['cfg']
    import concourse.bacc as bacc
    cfg = Cfg()
    nc = bacc.Bacc("TRN2", target_bir_lowering=False, debug=False,
                   num_devices=8)
    build(nc, cfg)
    nc.compile()
    _STATE['nc'], _STATE['cfg'] = nc, cfg
    return nc, cfg


def kernel(**inputs):
    from concourse import bass_utils
    x = np.ascontiguousarray(np.asarray(inputs['x'], dtype=np.float32))
    B, T, C = x.shape
    nc, cfg = get_compiled()
    sched = cfg.sched()
    w = {n: np.ascontiguousarray(np.asarray(inputs[n], dtype=np.float32))
         for n in W_NAMES}
    in_maps = []
    perms = []
    for core in range(8):
        b, pid = core // 2, core % 2
        perm = core_perm(pid, T)
        perms.append((b, perm))
        m = build_masks_np(perm, T, sched, cfg.NMASK)
        im = dict(w)
        im['xp'] = np.ascontiguousarray(x[b][perm])
        im['masks'] = m.astype(ml_dtypes.bfloat16)
        in_maps.append(im)
    res = bass_utils.run_bass_kernel_spmd(nc, in_maps,
                                          core_ids=list(range(8)),
                                          **_STATE.get('run_kwargs', {}))
    y = np.zeros((B, T, C), np.float32)
    for core in range(8):
        b, perm = perms[core]
        y[b][perm[:T // 2]] = res.results[core]['y']
    _STATE['last_result'] = res
    return y

